# revision 15
# baseline (speedup 1.0000x reference)
"""CapsuleNetwork Trainium2 kernel — 8-core data parallel, transposed dataflow.

kernel(**inputs) takes FULL unsharded inputs (np arrays keyed as in
setup_inputs()) and returns the FULL [4096, 100] logits.

Per-core dataflow (batch shard b=512; activations transposed [feature, batch];
class axis padded: class j owns 32 partitions of a 128-row tile (4 classes per
tile), the 16 class dims in the first half):
  x1T = relu(W1.T @ featT + b1)           bf16 matmuls, K padded to 10112
  x2T = relu(W2.T @ x1T + b2)             fp32r
  pT  = LayerNorm_i(WpFT.T @ x2T + bp)    grouped stats via indicator matmuls
  s0T = Wbar.T @ pn (bf16, streamed)      iteration-0 uniform-softmax shortcut
  dynamic routing (3 iters):
    squash: sig = Esq @ sT^2, phi = sig/((1+sig)sqrt(sig+eps))
    agreement: per jt: vtmp = phiE*sT; t2 = Wrm.T @ vtmp (PSUM);
      prod2 = t2*pn (DVE/Pool direct from PSUM, bf16 out); a = EI.T@prod2;
      blog (+)= a
    softmax+s (i-split): exN = exp(blog)*rZE;  cpI_i = exN * pnI0_i;
      sT = sum_i BDs_i.T @ cpI_i   (block-diagonal per-i tables, streamed)
  logits = Wcp.T @ (phiE*sT) + bc, PE transpose, DMA out.
"""
import sys

sys.path.insert(0, "/opt/trn_rl_repo")

from contextlib import ExitStack

import numpy as np
import ml_dtypes

import concourse.bass as bass
import concourse.tile as tile
from concourse import bacc, mybir
from concourse.bass_utils import run_bass_kernel_spmd

F32 = mybir.dt.float32
F32R = mybir.dt.float32r
BF16 = mybir.dt.bfloat16

N_CORES = 8
B = 512            # per-core batch
ID = 10000
KPAD = 10112       # 79 * 128
NK = KPAD // 128   # 79
H1, H2 = 512, 256
NP_, PD, NL, CD = 32, 8, 100, 16
NI = NP_ * PD      # 256
JP = 32
NJT = 25           # 25 tiles x [128 = 4 classes x 32]
LN_EPS = 1e-5
SQ_EPS = 1e-8

_CACHE = {}


def _round_fp32r(x):
    """fp32 -> fp32r: RNE to 11 mantissa bits (matches walrus fp32_to_fp32r)."""
    u = np.ascontiguousarray(x, dtype=np.float32).view(np.uint32)
    r = (u.astype(np.uint64) + 0x7FF + ((u >> 12) & 1)) & 0xFFFFF000
    return np.ascontiguousarray(r.astype(np.uint32).view(np.float32))


def _f32(x):
    return np.ascontiguousarray(np.asarray(x, dtype=np.float32))


def _f32r(x):
    return _round_fp32r(np.asarray(x, dtype=np.float32))


def _bf16(x):
    return np.ascontiguousarray(
        np.asarray(x, dtype=np.float32).astype(ml_dtypes.bfloat16))


def _part(x, k):
    """[K, M] with K=k*128 -> SBUF-friendly [128, k, M]."""
    K, M = x.shape
    return np.ascontiguousarray(x.reshape(k, 128, M).transpose(1, 0, 2))


def host_prep(W1, b1, W2, b2, Wp, bp, ln_g, ln_b, Wr, Wc, bc):
    H = {}
    W1p = np.zeros((KPAD, H1), np.float32)
    W1p[:ID] = np.asarray(W1, dtype=np.float32)
    H["W1"] = _bf16(W1p.reshape(NK, 128, H1))
    H["b1"] = _f32(np.asarray(b1).reshape(4, 128).T.reshape(128, 4, 1))
    H["W2"] = _f32r(_part(_f32(W2), 4))
    H["b2"] = _f32(np.asarray(b2).reshape(2, 128).T.reshape(128, 2, 1))
    H["WpFT"] = _f32r(_part(_f32(np.asarray(Wp).transpose(1, 0, 2).reshape(H2, NI)), 2))
    H["bpT"] = _f32(np.asarray(bp).reshape(2, 128).T.reshape(128, 2, 1))
    H["gT"] = _f32(np.asarray(ln_g).reshape(2, 128).T.reshape(128, 2, 1))
    H["lbT"] = _f32(np.asarray(ln_b).reshape(2, 128).T.reshape(128, 2, 1))
    Wr = np.asarray(Wr, dtype=np.float32)

    # Wbar (streamed): [25 jt][2 ch][128 K(n,i), 128 M(4j x 32)] with 1/NL folded
    Wbar = np.zeros((NJT, 2, 128, 128), np.float32)
    # Wrm (resident bf16): [128 K(4j x 32 o), 2 ch, 25 jt, 128 M(n,i)]
    Wrm = np.zeros((128, 2, NJT, 128), np.float32)
    for jt in range(NJT):
        for g in range(4):
            j = 4 * jt + g
            m_io = Wr[:, j].transpose(0, 2, 1).reshape(NI, CD)   # [(n,i), o]
            for ch in range(2):
                blk = m_io[128 * ch: 128 * (ch + 1)]             # [128, 16]
                Wbar[jt, ch, :, 32 * g: 32 * g + CD] = blk / NL
                Wrm[32 * g: 32 * g + CD, ch, jt, :] = blk.T
    H["Wbar"] = _bf16(Wbar)
    H["Wrm"] = _bf16(Wrm)

    # BDs (streamed, i-split class-block-diagonal):
    # [25 jt][128 K(4g x 32 n)][8 i][128 M(4g x 32 o-slots)]
    # BDs[jt, 32g+n, i, 32g+o] = Wr[n, 4jt+g, o, i]
    BDs = np.zeros((NJT, 128, PD, 128), np.float32)
    for jt in range(NJT):
        for g in range(4):
            j = 4 * jt + g
            for i in range(PD):
                BDs[jt, 32 * g: 32 * g + NP_, i, 32 * g: 32 * g + CD] = \
                    Wr[:, j, :, i]
    H["BDs"] = _bf16(BDs)

    # EIrep: pn -> pnI0 expansion.  pnI0_i[(4g x 32 n), b] = pn[(n,i), b].
    # [128 K((n',i') chunk c), 2 ch, 8 i, 128 M(4g x 32 n)]
    EIrep = np.zeros((128, 2, PD, 128), np.float32)
    for c in range(2):
        for n in range(16 * c, 16 * (c + 1)):
            for i in range(PD):
                k = PD * (n - 16 * c) + i
                for g in range(4):
                    EIrep[k, c, i, 32 * g + n] = 1.0
    H["EIrep"] = _bf16(EIrep)

    # En32: [32 K(n), 128 M(4g x 32 n)] replicate Z over the 4 class slots
    En32 = np.zeros((NP_, 128), np.float32)
    for n in range(NP_):
        for g in range(4):
            En32[n, 32 * g + n] = 1.0
    H["En32"] = _bf16(En32)

    E8m = np.zeros((NI, NP_), np.float32)    # mean over i (1/8 folded)
    Eexp8 = np.zeros((NP_, NI), np.float32)  # expand n -> (n,i)
    for n in range(NP_):
        E8m[n * PD: (n + 1) * PD, n] = 1.0 / PD
        Eexp8[n, n * PD: (n + 1) * PD] = 1.0
    H["E8m"] = _f32r(_part(E8m, 2))          # [128, 2, 32]
    H["Eexp8"] = _f32r(Eexp8)                # [32, 256]
    H["EI"] = _bf16(_part(Eexp8.T, 2))       # [128, 2, 32] sum over i (bf16)

    Esq = np.zeros((128, NJT, NL), np.float32)
    Eexp32 = np.zeros((NL, NJT, 128), np.float32)
    Esum = np.zeros((128, NJT, NP_), np.float32)
    for jt in range(NJT):
        for g in range(4):
            Esq[32 * g: 32 * g + CD, jt, 4 * jt + g] = 1.0
            Eexp32[4 * jt + g, jt, 32 * g: 32 * (g + 1)] = 1.0
            Esum[32 * g: 32 * (g + 1), jt, :] = np.eye(NP_)
    H["Esq"] = _bf16(Esq)
    H["Eexp32"] = _bf16(Eexp32)
    H["Esum"] = _bf16(Esum)

    Wc = np.asarray(Wc, dtype=np.float32)
    PJ = NL * JP
    Wcp = np.zeros((PJ, NL), np.float32)
    for j in range(NL):
        Wcp[j * JP: j * JP + CD] = Wc[j * CD: (j + 1) * CD]
    H["Wcp"] = _bf16(Wcp.reshape(NJT, 128, NL))
    H["bc"] = _f32(np.asarray(bc).reshape(NL, 1))
    H["ident"] = _f32(np.eye(128))
    return H


def build():
    nc = bacc.Bacc("TRN2", target_bir_lowering=False, debug=False)
    d = {}

    def din(name, shape, dt=F32):
        d[name] = nc.dram_tensor(name, list(shape), dt, kind="ExternalInput").ap()

    din("featT", (NK, 128, B), BF16)
    din("W1", (NK, 128, H1), BF16)
    din("b1", (128, 4, 1)); din("W2", (128, 4, H2), F32R); din("b2", (128, 2, 1))
    din("WpFT", (128, 2, NI), F32R)
    din("bpT", (128, 2, 1)); din("gT", (128, 2, 1)); din("lbT", (128, 2, 1))
    din("Wbar", (NJT, 2, 128, 128), BF16)
    din("Wrm", (128, 2, NJT, 128), BF16)
    din("BDs", (NJT, 128, PD, 128), BF16)
    din("EIrep", (128, 2, PD, 128), BF16)
    din("En32", (NP_, 128), BF16)
    din("E8m", (128, 2, NP_), F32R); din("Eexp8", (NP_, NI), F32R)
    din("EI", (128, 2, NP_), BF16)
    din("Esq", (128, NJT, NL), BF16); din("Eexp32", (NL, NJT, 128), BF16)
    din("Esum", (128, NJT, NP_), BF16)
    din("Wcp", (NJT, 128, NL), BF16); din("bc", (NL, 1))
    din("ident", (128, 128))
    out = nc.dram_tensor("logits", [B, NL], F32, kind="ExternalOutput").ap()

    AF = mybir.ActivationFunctionType
    OP = mybir.AluOpType

    with tile.TileContext(nc) as tc, ExitStack() as ctx:
        const = ctx.enter_context(tc.tile_pool(name="const", bufs=1))

        def cload(name, dt=F32):
            src = d[name]
            t = const.tile(list(src.shape), dt, tag=name)
            nc.sync.dma_start(t[:], src)
            return t

        b1_sb = cload("b1"); w2_sb = cload("W2", F32R); b2_sb = cload("b2")
        wpft_sb = cload("WpFT", F32R); bpt_sb = cload("bpT")
        gt_sb = cload("gT"); lbt_sb = cload("lbT")
        wrm_sb = cload("Wrm", BF16)
        eirep_sb = cload("EIrep", BF16); en32_sb = cload("En32", BF16)
        e8m_sb = cload("E8m", F32R); eexp8_sb = cload("Eexp8", F32R)
        ei_sb = cload("EI", BF16)
        esq_sb = cload("Esq", BF16); eexp32_sb = cload("Eexp32", BF16)
        esum_sb = cload("Esum", BF16)
        bc_sb = cload("bc"); ident_sb = cload("ident")
        eps_ln = const.tile([128, 1], F32, tag="epsln")
        nc.gpsimd.memset(eps_ln[:], LN_EPS)
        eps_sq = const.tile([128, 1], F32, tag="epssq")
        nc.gpsimd.memset(eps_sq[:], SQ_EPS)

        state = ctx.enter_context(tc.tile_pool(name="state", bufs=1))
        x1T = state.tile([128, 4, B], F32R, tag="x1T")
        x2T = state.tile([128, 2, B], F32R, tag="x2T")
        pn_b = state.tile([128, 2 * B], BF16, tag="pn_b")
        pnI0 = state.tile([128, PD, B], BF16, tag="pnI0")
        blog = state.tile([128, NJT, B], F32, tag="blog")
        sT = state.tile([128, NJT, B], BF16, tag="sT")
        sig = state.tile([NL, B], F32, tag="sig")
        phi = state.tile([NL, B], BF16, tag="phi")
        pnIr = state.tile([128, PD, B], BF16, tag="pnIr")

        # ---------- Stage A: x1T = relu(W1.T @ featT + b1) ----------
        with tc.tile_pool(name="psA", bufs=1, space="PSUM") as psA, \
             tc.tile_pool(name="streamA", bufs=4) as sa:
            x1_ps = [psA.tile([128, B], F32, tag=f"x1ps{h}", name=f"x1ps{h}")
                     for h in range(4)]
            for k in range(NK):
                ft = sa.tile([128, B], BF16, tag="ft")
                nc.sync.dma_start(ft[:], d["featT"][k])
                wt = sa.tile([128, H1], BF16, tag="wt")
                nc.sync.dma_start(wt[:], d["W1"][k])
                for h in range(4):
                    nc.tensor.matmul(
                        x1_ps[h][:], wt[:, bass.ts(h, 128)], ft[:],
                        start=(k == 0), stop=(k == NK - 1))
            for h in range(4):
                nc.scalar.activation(x1T[:, h, :], x1_ps[h][:], AF.Relu,
                                     bias=b1_sb[:, h, :], scale=1.0)

        # ---------- Stage B + C + LayerNorm ----------
        with tc.tile_pool(name="psB", bufs=2, space="PSUM") as psB, \
             tc.tile_pool(name="psBa", bufs=1, space="PSUM") as psBa, \
             tc.tile_pool(name="scrB", bufs=2) as scrB:
            for c in range(2):
                x2_ps = psB.tile([128, B], F32, tag="mm")
                for k in range(4):
                    nc.tensor.matmul(
                        x2_ps[:], w2_sb[:, k, bass.ts(c, 128)], x1T[:, k, :],
                        start=(k == 0), stop=(k == 3))
                nc.scalar.activation(x2T[:, c, :], x2_ps[:], AF.Relu,
                                     bias=b2_sb[:, c, :], scale=1.0)

            praw = scrB.tile([128, 2, B], F32R, tag="praw", bufs=1)
            for c in range(2):
                p_ps = psB.tile([128, B], F32, tag="mm")
                for k in range(2):
                    nc.tensor.matmul(
                        p_ps[:], wpft_sb[:, k, bass.ts(c, 128)], x2T[:, k, :],
                        start=(k == 0), stop=(k == 1))
                nc.scalar.activation(praw[:, c, :], p_ps[:], AF.Identity,
                                     bias=bpt_sb[:, c, :], scale=1.0)

            mu_ps = psBa.tile([NP_, B], F32, tag="acc")
            for c in range(2):
                nc.tensor.matmul(mu_ps[:], e8m_sb[:, c, :], praw[:, c, :],
                                 start=(c == 0), stop=(c == 1))
            mu_sb = scrB.tile([NP_, B], F32R, tag="mu")
            nc.scalar.activation(mu_sb[:], mu_ps[:], AF.Copy, bias=0.0, scale=1.0)

            q = scrB.tile([128, 2, B], F32, tag="q", bufs=1)
            var_ps = psBa.tile([NP_, B], F32, tag="acc2")
            for c in range(2):
                me_ps = psB.tile([128, B], F32, tag="mm")
                nc.tensor.matmul(me_ps[:], eexp8_sb[:, bass.ts(c, 128)],
                                 mu_sb[:], start=True, stop=True)
                nc.vector.tensor_sub(q[:, c, :], praw[:, c, :].bitcast(F32),
                                     me_ps[:])
                qsq = scrB.tile([128, B], F32R, tag="qsq")
                nc.scalar.activation(qsq[:], q[:, c, :], AF.Square,
                                     bias=0.0, scale=1.0)
                nc.tensor.matmul(var_ps[:], e8m_sb[:, c, :], qsq[:],
                                 start=(c == 0), stop=(c == 1))
            sd_sb = scrB.tile([NP_, B], F32R, tag="sd")
            nc.scalar.activation(sd_sb[:], var_ps[:], AF.Sqrt,
                                 bias=eps_ln[:NP_, :], scale=1.0)
            for c in range(2):
                se_ps = psB.tile([NI // 2, B], F32, tag="mm")
                nc.tensor.matmul(se_ps[:], eexp8_sb[:, bass.ts(c, 128)],
                                 sd_sb[:], start=True, stop=True)
                rstd_exp = scrB.tile([128, B], F32, tag="rstd")
                nc.vector.reciprocal(rstd_exp[:], se_ps[:])
                qr = scrB.tile([128, B], F32, tag="qr")
                nc.vector.tensor_mul(qr[:], q[:, c, :], rstd_exp[:])
                nc.scalar.activation(pn_b[:, bass.ts(c, B)], qr[:],
                                     AF.Identity,
                                     bias=lbt_sb[:, c, :],
                                     scale=gt_sb[:, c, :])

            # pnI0_i[(4g x 32 n), b] = pn[(n,i), b]  (static, bf16)
            for i in range(PD):
                pi_ps = psB.tile([128, B], F32, tag="mm")
                for c in range(2):
                    nc.tensor.matmul(pi_ps[:], eirep_sb[:, c, i, :],
                                     pn_b[:, bass.ts(c, B)],
                                     start=(c == 0), stop=(c == 1))
                nc.vector.tensor_copy(pnI0[:, i, :], pi_ps[:])

        # ---------- s0 = Wbar.T @ pn (streamed) ----------
        with tc.tile_pool(name="psS0", bufs=2, space="PSUM") as psS0, \
             tc.tile_pool(name="streamW", bufs=4) as sw:
            for jt in range(NJT):
                s_ps = psS0.tile([128, B], F32, tag="s")
                for c in range(2):
                    wb = sw.tile([128, 128], BF16, tag="wb")
                    nc.sync.dma_start(wb[:], d["Wbar"][jt, c])
                    nc.tensor.matmul(s_ps[:], wb[:], pn_b[:, bass.ts(c, B)],
                                     start=(c == 0), stop=(c == 1))
                nc.scalar.activation(sT[:, jt, :], s_ps[:], AF.Copy,
                                     bias=0.0, scale=1.0)

        # ---------- routing helpers ----------
        def emit_squash():
            """sT -> sig -> phi  (no vT materialization; phi applied later)."""
            with tc.tile_pool(name="psQ", bufs=1, space="PSUM") as psQa, \
                 tc.tile_pool(name="scrQ", bufs=3) as scrQ:
                sig_ps = psQa.tile([NL, B], F32, tag="sig")
                for jt in range(NJT):
                    sq = scrQ.tile([128, B], BF16, tag="sq")
                    nc.vector.tensor_mul(sq[:], sT[:, jt, :], sT[:, jt, :])
                    nc.tensor.matmul(sig_ps[:], esq_sb[:, jt, :], sq[:],
                                     start=(jt == 0), stop=(jt == NJT - 1))
                nc.scalar.activation(sig[:], sig_ps[:], AF.Copy,
                                     bias=0.0, scale=1.0)
                u = scrQ.tile([NL, B], F32, tag="u")
                nc.scalar.activation(u[:], sig[:], AF.Sqrt,
                                     bias=eps_sq[:NL, :], scale=1.0)
                w = scrQ.tile([NL, B], F32, tag="w")
                nc.vector.scalar_tensor_tensor(w[:], sig[:], 1.0, u[:],
                                               op0=OP.add, op1=OP.mult)
                wr = scrQ.tile([NL, B], F32, tag="wr")
                nc.vector.reciprocal(wr[:], w[:])
                nc.vector.tensor_mul(phi[:], sig[:], wr[:])

        def emit_agreement(first):
            """blog (+)= a:  vtmp = phiE*sT ; t2 = Wrm.T @ vtmp ;
            prod2 = t2*pn (DVE/Pool from PSUM) ; a = EI.T @ prod2."""
            with tc.tile_pool(name="psG", bufs=2, space="PSUM") as psGa, \
                 tc.tile_pool(name="psGt", bufs=2, space="PSUM") as psGt, \
                 tc.tile_pool(name="psGf", bufs=2, space="PSUM") as psGf, \
                 tc.tile_pool(name="scrG", bufs=3) as scrG:
                for jt in range(NJT):
                    a_ps = psGa.tile([128, B], F32, tag="a")
                    for gp in range(2):
                        t2s = []
                        for gg in range(2):
                            g = 2 * gp + gg
                            t2 = psGt.tile([128, 2 * B], F32, tag=f"t{gg}",
                                           bufs=1)
                            for c in range(2):
                                nc.tensor.matmul(
                                    t2[:, bass.ts(c, B)],
                                    wrm_sb[:, c, jt, :][32 * g: 32 * g + CD, :],
                                    sT[:, jt, :][32 * g: 32 * g + CD, :],
                                    start=True, stop=True,
                                    tile_position=(32 * g, 0))
                            t2s.append((g, t2))
                        for (g, t2) in t2s:
                            prod2 = scrG.tile([128, 2 * B], BF16, tag="prod2")
                            if g < 3:
                                te2 = scrG.tile([128, 2 * B], BF16, tag="te2")
                                nc.scalar.activation(te2[:], t2[:], AF.Copy,
                                                     bias=0.0, scale=1.0)
                                nc.vector.tensor_mul(prod2[:], te2[:],
                                                     pn_b[:])
                            else:
                                nc.vector.tensor_mul(prod2[:], t2[:], pn_b[:])
                            for c in range(2):
                                nc.tensor.matmul(
                                    a_ps[32 * g: 32 * (g + 1), :],
                                    ei_sb[:, c, :], prod2[:, bass.ts(c, B)],
                                    start=(c == 0), stop=(c == 1),
                                    tile_position=(0, 32 * g))
                    # phi enters only here: blog (+)= phiE * a
                    fe_ps = psGf.tile([128, B], F32, tag="fe")
                    nc.tensor.matmul(fe_ps[:], eexp32_sb[:, jt, :], phi[:],
                                     start=True, stop=True)
                    feB = scrG.tile([128, B], BF16, tag="feB")
                    nc.scalar.activation(feB[:], fe_ps[:], AF.Copy,
                                         bias=0.0, scale=1.0)
                    if first:
                        nc.vector.tensor_mul(blog[:, jt, :], a_ps[:], feB[:])
                    else:
                        atmp = scrG.tile([128, B], F32, tag="atmp")
                        nc.vector.tensor_mul(atmp[:], a_ps[:], feB[:])
                        nc.vector.tensor_add(blog[:, jt, :], blog[:, jt, :],
                                             atmp[:])

        def emit_softmax_and_s():
            """Z = sum_j exp(blog) ; rZE = 1/Z expanded ;
            per jt: exN = exp(blog)*rZE ; cpI_i = exN*pnI0_i ;
            sT = sum_i BDs_i.T @ cpI_i  (streamed block-diag tables)."""
            with tc.tile_pool(name="psX", bufs=1, space="PSUM") as psXa, \
                 tc.tile_pool(name="psXs", bufs=2, space="PSUM") as psXs, \
                 tc.tile_pool(name="streamB", bufs=3) as sb, \
                 tc.tile_pool(name="scrX", bufs=3) as scrX:
                sum_ps = psXa.tile([NP_, B], F32, tag="sum")
                for jt in range(NJT):
                    ex = scrX.tile([128, B], BF16, tag="ex")
                    nc.scalar.activation(ex[:], blog[:, jt, :], AF.Exp,
                                         bias=0.0, scale=1.0)
                    nc.tensor.matmul(sum_ps[:], esum_sb[:, jt, :], ex[:],
                                     start=(jt == 0), stop=(jt == NJT - 1))
                rz = scrX.tile([NP_, B], F32, tag="rz")
                nc.vector.reciprocal(rz[:], sum_ps[:])
                rzb = scrX.tile([NP_, B], BF16, tag="rzb")
                nc.scalar.activation(rzb[:], rz[:], AF.Copy, bias=0.0,
                                     scale=1.0)
                rze_ps = psXa.tile([128, B], F32, tag="rze")
                nc.tensor.matmul(rze_ps[:], en32_sb[:], rzb[:],
                                 start=True, stop=True)
                for i in range(PD):
                    nc.vector.tensor_mul(pnIr[:, i, :], pnI0[:, i, :],
                                         rze_ps[:])
                for jt in range(NJT):
                    bd = sb.tile([128, PD, 128], BF16, tag="bd")
                    nc.sync.dma_start(bd[:], d["BDs"][jt])
                    ex = scrX.tile([128, B], BF16, tag="ex")
                    nc.scalar.activation(ex[:], blog[:, jt, :], AF.Exp,
                                         bias=0.0, scale=1.0)
                    s_ps = psXs.tile([128, B], F32, tag="s")
                    cps = scrX.tile([128, PD, B], BF16, tag="cp")
                    for i in range(PD):
                        if i < 4:
                            nc.gpsimd.tensor_mul(cps[:, i, :], ex[:],
                                                 pnIr[:, i, :])
                        else:
                            nc.vector.tensor_mul(cps[:, i, :], ex[:],
                                                 pnIr[:, i, :])
                        nc.tensor.matmul(s_ps[:], bd[:, i, :], cps[:, i, :],
                                         start=(i == 0), stop=(i == PD - 1))
                    nc.scalar.activation(sT[:, jt, :], s_ps[:], AF.Copy,
                                         bias=0.0, scale=1.0)

        # ---------- routing ----------
        emit_squash()
        emit_agreement(first=True)
        emit_softmax_and_s()
        emit_squash()
        emit_agreement(first=False)
        emit_softmax_and_s()
        emit_squash()

        # ---------- logits (fold phi in per-tile) ----------
        with tc.tile_pool(name="psL", bufs=1, space="PSUM") as psLa, \
             tc.tile_pool(name="psLm", bufs=2, space="PSUM") as psLm, \
             tc.tile_pool(name="streamC", bufs=4) as scc, \
             tc.tile_pool(name="scrL", bufs=2) as scrL:
            lg_ps = psLa.tile([NL, B], F32, tag="lg")
            for jt in range(NJT):
                fe_ps = psLm.tile([128, B], F32, tag="fe")
                nc.tensor.matmul(fe_ps[:], eexp32_sb[:, jt, :], phi[:],
                                 start=True, stop=True)
                vtmp = scrL.tile([128, B], BF16, tag="vt")
                nc.vector.tensor_mul(vtmp[:], sT[:, jt, :], fe_ps[:])
                wc = scc.tile([128, NL], BF16, tag="wc")
                nc.sync.dma_start(wc[:], d["Wcp"][jt])
                nc.tensor.matmul(lg_ps[:], wc[:], vtmp[:],
                                 start=(jt == 0), stop=(jt == NJT - 1))
            lg_sb = scrL.tile([NL, B], F32, tag="lgsb")
            nc.scalar.activation(lg_sb[:], lg_ps[:], AF.Identity,
                                 bias=bc_sb[:], scale=1.0)
            for bt in range(4):
                tr_ps = psLm.tile([128, NL], F32, tag="tr")
                nc.tensor.transpose(tr_ps[:], lg_sb[:, bass.ts(bt, 128)],
                                    ident_sb[:NL, :NL])
                og = scrL.tile([128, NL], F32, tag="og")
                nc.vector.tensor_copy(og[:], tr_ps[:])
                nc.sync.dma_start(out[bass.ts(bt, 128), :], og[:])

    nc.compile()
    return nc


def kernel(features, W1, b1, W2, b2, Wp, bp, ln_g, ln_b, Wr, Wc, bc):
    features = np.ascontiguousarray(np.asarray(features, dtype=np.float32))
    H = host_prep(W1, b1, W2, b2, Wp, bp, ln_g, ln_b, Wr, Wc, bc)
    ins = {k: H[k] for k in [
        "W1", "b1", "W2", "b2", "WpFT", "bpT", "gT", "lbT", "Wbar", "Wrm",
        "BDs", "EIrep", "En32", "E8m", "Eexp8", "EI", "Esq", "Eexp32", "Esum",
        "Wcp", "bc", "ident"]}
    if "nc" not in _CACHE:
        _CACHE["nc"] = build()
    nc = _CACHE["nc"]

    in_maps = []
    for c in range(N_CORES):
        sl = features[c * B: (c + 1) * B]
        fT = np.zeros((KPAD, B), np.float32)
        fT[:ID] = sl.T
        in_maps.append({"featT": _bf16(fT).reshape(NK, 128, B), **ins})

    res = run_bass_kernel_spmd(nc, in_maps, list(range(N_CORES)))
    _CACHE["last_results"] = res
    return np.concatenate([res.results[c]["logits"] for c in range(N_CORES)],
                          axis=0)
